# revision 5
# baseline (speedup 1.0000x reference)
"""Expert-mixture (top-1 MoE) Trainium2 kernel, expert-parallel across 8 cores.

Strategy:
  - Host computes the router (x @ Wr + br, argmax) and dispatches tokens:
    all tokens routed to expert e are gathered, transposed, and padded to a
    fixed capacity, forming core e's shard ("all-to-all dispatch by argmax
    topic" done at shard time, since kernel() receives full inputs on host).
  - Core e computes hT = relu(W1[e].T @ xT + b1[e]) followed by
    outT = W2[e].T @ h, entirely on-device (TensorE GEMMs via Tile).
  - Host scatters each expert's rows back into the full [B, C] output and
    adds b2[topic] (bias add commutes with the gather).

Per-core device layout (SPMD, one program):
  xt  [D, CAP]  f32   token block, transposed, zero-padded
  w1  [D, H]    f32   W1[e] (native layout == lhsT chunks)
  b1t [128, 16] f32   b1[e] rearranged so column m = b1[m*128:(m+1)*128]
  w2t [128, 48] f32   W2[e] rearranged so [:, 3m:3m+3] = W2[e][128m:128(m+1)]
  ot  [3, CAP]  f32   output, transposed
"""

import numpy as np

import concourse.bass as bass
import concourse.mybir as mybir
import concourse.tile as tile
from concourse import bacc
from concourse.bass_utils import run_bass_kernel_spmd

B, D, H, E, C = 16384, 1024, 2048, 8, 3
N_CORES = 8
P = 128
KD = D // P    # 8 contraction chunks for GEMM1
MH = H // P    # 16 H chunks
TB = 512       # token block (matmul moving dim)
CAP = 2560     # per-expert token capacity (mean 2048, ~12 sigma headroom)

MM_DTYPE = mybir.dt.float32r  # PE compute dtype (f32 data, full-rate mode)

_nc_cache: dict = {}


def build_nc(cap: int, reps: int = 1, mm_dtype=MM_DTYPE):
    """Build + compile the SPMD program. reps>1 wraps the body in a device
    loop (for steady-state timing); data loads stay inside the loop so each
    iteration models one cold kernel execution."""
    nt = cap // TB
    assert cap % TB == 0

    nc = bacc.Bacc("TRN2", target_bir_lowering=False, debug=False,
                   num_devices=N_CORES)
    f32 = mybir.dt.float32
    # Matmul operands live in mm_dtype end-to-end (DRAM included): for
    # float32r (TF32) the host pre-rounds, so DMA is a pure move and the
    # BIR verifier's "rounded producer" requirement is satisfied at the
    # ExternalInput boundary.
    xt = nc.dram_tensor("xt", [D, cap], mm_dtype, kind="ExternalInput").ap()
    w1 = nc.dram_tensor("w1", [D, H], mm_dtype, kind="ExternalInput").ap()
    b1t = nc.dram_tensor("b1t", [P, MH], f32, kind="ExternalInput").ap()
    w2t = nc.dram_tensor("w2t", [P, MH * C], mm_dtype, kind="ExternalInput").ap()
    ot = nc.dram_tensor("ot", [C, cap], f32, kind="ExternalOutput").ap()

    with tile.TileContext(nc) as tc:
        with (
            tc.tile_pool(name="w1p", bufs=1) as w1p,
            tc.tile_pool(name="xtp", bufs=1) as xtp,
            tc.tile_pool(name="cst", bufs=1) as cst,
            tc.tile_pool(name="htp", bufs=1) as htp,
            tc.tile_pool(name="o2p", bufs=1) as o2p,
            tc.tile_pool(name="ps", bufs=1, space="PSUM") as psp,
        ):
            def body(_iv=None):
                b1_sb = cst.tile([P, MH], f32, tag="b1")
                nc.sync.dma_start(b1_sb[:], b1t[:])
                w2_sb = cst.tile([P, MH * C], mm_dtype, tag="w2")
                nc.sync.dma_start(w2_sb[:], w2t[:])

                w1_sb = []
                xt_sb = []
                for k in range(KD):
                    wt = w1p.tile([P, H], mm_dtype, tag=f"w1k{k}")
                    nc.sync.dma_start(wt[:], w1[k * P:(k + 1) * P, :])
                    w1_sb.append(wt)
                    xtile = xtp.tile([P, cap], mm_dtype, tag=f"xtk{k}")
                    nc.sync.dma_start(xtile[:], xt[k * P:(k + 1) * P, :])
                    xt_sb.append(xtile)

                o2_sb = o2p.tile([C, cap], f32, tag="o2")

                for t in range(nt):
                    ht_tiles = []
                    for m in range(MH):
                        ps1 = psp.tile([P, TB], f32, tag="ps1", bufs=4)
                        for k in range(KD):
                            nc.tensor.matmul(
                                ps1[:],
                                w1_sb[k][:, m * P:(m + 1) * P],
                                xt_sb[k][:, t * TB:(t + 1) * TB],
                                start=(k == 0),
                                stop=(k == KD - 1),
                            )
                        ht = htp.tile([P, TB], mm_dtype, tag=f"ht{m}")
                        nc.scalar.activation(
                            ht[:], ps1[:],
                            mybir.ActivationFunctionType.Relu,
                            bias=b1_sb[:, m:m + 1],
                        )
                        ht_tiles.append(ht)

                    ps2 = psp.tile([C, TB], f32, tag="ps2", bufs=2)
                    for m in range(MH):
                        nc.tensor.matmul(
                            ps2[:],
                            w2_sb[:, m * C:(m + 1) * C],
                            ht_tiles[m][:],
                            start=(m == 0),
                            stop=(m == MH - 1),
                        )
                    nc.vector.tensor_copy(o2_sb[:, t * TB:(t + 1) * TB], ps2[:])

                nc.sync.dma_start(ot[:], o2_sb[:])

            if reps == 1:
                body()
            else:
                hints = (mybir.EngineType.PE, mybir.EngineType.SP,
                         mybir.EngineType.Activation, mybir.EngineType.DVE)
                with tc.For_i(0, reps, 1, hint_engines=hints) as iv:
                    body(iv)

    nc.compile()
    return nc


def _get_nc(cap: int):
    key = (cap, MM_DTYPE)
    if key not in _nc_cache:
        _nc_cache[key] = build_nc(cap)
    return _nc_cache[key]


def _expert_mlp_host(xr, W1e, b1e, W2e, b2e):
    h = np.maximum(xr.astype(np.float32) @ W1e + b1e, 0.0)
    return h @ W2e + b2e


def _tf32_round(a: np.ndarray) -> np.ndarray:
    """Round f32 to TF32 (10-bit mantissa), round-to-nearest-even."""
    if MM_DTYPE != mybir.dt.float32r:
        return a
    b = np.ascontiguousarray(a, dtype=np.float32).copy().view(np.uint32)
    b += 0x00000FFF + ((b >> 13) & 1)
    b &= np.uint32(0xFFFFE000)
    return b.view(np.float32)


def make_in_maps(x, W1, b1, W2, idx, cap):
    in_maps = []
    for e in range(E):
        ie = idx[e][:cap]
        xtc = np.zeros((D, cap), dtype=np.float32)
        xtc[:, :len(ie)] = x[ie].T
        in_maps.append({
            "xt": _tf32_round(xtc),
            "w1": _tf32_round(W1[e]),
            "b1t": np.ascontiguousarray(b1[e].reshape(MH, P).T),
            "w2t": _tf32_round(
                W2[e].reshape(MH, P, C).transpose(1, 0, 2).reshape(P, MH * C)),
        })
    return in_maps


def kernel(x, Wr, br, W1, b1, W2, b2):
    x = np.asarray(x, dtype=np.float32)
    Wr = np.asarray(Wr, dtype=np.float32)
    br = np.asarray(br, dtype=np.float32)
    W1 = np.asarray(W1, dtype=np.float32)
    b1 = np.asarray(b1, dtype=np.float32)
    W2 = np.asarray(W2, dtype=np.float32)
    b2 = np.asarray(b2, dtype=np.float32)

    # Router on host: this decides the (expert-parallel) sharding.
    logits = x @ Wr + br
    topics = np.argmax(logits, axis=1)

    idx = [np.flatnonzero(topics == e) for e in range(E)]
    # Keep the NEFF shape fixed; if an expert ever exceeds CAP (astronomically
    # rare for the target distribution) the overflow rows are computed on host.
    cap = CAP
    in_maps = make_in_maps(x, W1, b1, W2, idx, cap)
    nc = _get_nc(cap)
    res = run_bass_kernel_spmd(nc, in_maps, core_ids=list(range(N_CORES)))

    out = np.empty((B, C), dtype=np.float32)
    for e in range(E):
        ie = idx[e][:cap]
        out[ie] = res.results[e]["ot"][:, :len(ie)].T + b2[e]
        if len(idx[e]) > cap:
            ov = idx[e][cap:]
            out[ov] = _expert_mlp_host(x[ov], W1[e], b1[e], W2[e], b2[e])
    return out


# revision 8
# speedup vs baseline: 1.2441x; 1.2441x over previous
"""Expert-mixture (top-1 MoE) Trainium2 kernel, expert-parallel across 8 cores.

Strategy:
  - Host computes the router (x @ Wr + br, argmax) and dispatches tokens:
    all tokens routed to expert e are gathered, transposed, and padded to a
    fixed capacity, forming core e's shard ("all-to-all dispatch by argmax
    topic" done at shard time, since kernel() receives full inputs on host).
  - Core e computes hT = relu(W1[e].T @ xT + b1[e]) followed by
    outT = W2[e].T @ h, entirely on-device (TensorE GEMMs via Tile).
  - Host scatters each expert's rows back into the full [B, C] output and
    adds b2[topic] (bias add commutes with the gather).

Per-core device layout (SPMD, one program):
  xt  [D, CAP]  f32   token block, transposed, zero-padded
  w1  [D, H]    f32   W1[e] (native layout == lhsT chunks)
  b1t [128, 16] f32   b1[e] rearranged so column m = b1[m*128:(m+1)*128]
  w2t [128, 48] f32   W2[e] rearranged so [:, 3m:3m+3] = W2[e][128m:128(m+1)]
  ot  [3, CAP]  f32   output, transposed
"""

import numpy as np

import concourse.bass as bass
import concourse.mybir as mybir
import concourse.tile as tile
from concourse import bacc
from concourse.bass_utils import run_bass_kernel_spmd

B, D, H, E, C = 16384, 1024, 2048, 8, 3
N_CORES = 8
P = 128
KD = D // P    # 8 contraction chunks for GEMM1
MH = H // P    # 16 H chunks
TB = 512       # token block (matmul moving dim)
CAP = 2304     # per-expert token capacity (mean 2048, ~6 sigma headroom)

MM_DTYPE = mybir.dt.float32r  # PE compute dtype (f32 data, full-rate mode)

_nc_cache: dict = {}


def build_nc(cap: int, reps: int = 1, mm_dtype=MM_DTYPE):
    """Build + compile the SPMD program. reps>1 wraps the body in a device
    loop (for steady-state timing); data loads stay inside the loop so each
    iteration models one cold kernel execution."""
    # Token blocks: TB-sized, last may be ragged (min 256 keeps fp32r matmuls
    # at full rate).
    blocks = []
    off = 0
    while off < cap:
        size = min(TB, cap - off)
        blocks.append((off, size))
        off += size
    assert all(s >= 256 for _, s in blocks)

    nc = bacc.Bacc("TRN2", target_bir_lowering=False, debug=False,
                   num_devices=N_CORES)
    f32 = mybir.dt.float32
    # Matmul operands live in mm_dtype end-to-end (DRAM included): for
    # float32r (TF32) the host pre-rounds, so DMA is a pure move and the
    # BIR verifier's "rounded producer" requirement is satisfied at the
    # ExternalInput boundary.
    xt = nc.dram_tensor("xt", [D, cap], mm_dtype, kind="ExternalInput").ap()
    w1 = nc.dram_tensor("w1", [D, H], mm_dtype, kind="ExternalInput").ap()
    b1t = nc.dram_tensor("b1t", [P, MH], f32, kind="ExternalInput").ap()
    w2t = nc.dram_tensor("w2t", [P, MH * C], mm_dtype, kind="ExternalInput").ap()
    ot = nc.dram_tensor("ot", [C, cap], f32, kind="ExternalOutput").ap()

    with tile.TileContext(nc) as tc:
        with (
            tc.tile_pool(name="w1p", bufs=1) as w1p,
            tc.tile_pool(name="xtp", bufs=1) as xtp,
            tc.tile_pool(name="cst", bufs=1) as cst,
            tc.tile_pool(name="htp", bufs=1) as htp,
            tc.tile_pool(name="o2p", bufs=1) as o2p,
            tc.tile_pool(name="ps", bufs=1, space="PSUM") as psp,
        ):
            def body(_iv=None):
                b1_sb = cst.tile([P, MH], f32, tag="b1")
                nc.sync.dma_start(b1_sb[:], b1t[:])
                w2_sb = cst.tile([P, MH * C], mm_dtype, tag="w2")
                nc.sync.dma_start(w2_sb[:], w2t[:])

                w1_sb = []
                xt_sb = []
                for k in range(KD):
                    wt = w1p.tile([P, H], mm_dtype, tag=f"w1k{k}")
                    nc.sync.dma_start(wt[:], w1[k * P:(k + 1) * P, :])
                    w1_sb.append(wt)
                    xtile = xtp.tile([P, cap], mm_dtype, tag=f"xtk{k}")
                    nc.sync.dma_start(xtile[:], xt[k * P:(k + 1) * P, :])
                    xt_sb.append(xtile)

                o2_sb = o2p.tile([C, cap], f32, tag="o2")

                # GEMM1 runs k-outer within groups of 8 H-chunks (8 PSUM
                # banks): the first matmuls only need chunk k=0 of W1/xt, so
                # compute overlaps the remaining weight/activation DMA
                # instead of stalling ~50us on the full 18MB.
                for t, (toff, tsz) in enumerate(blocks):
                    ht_tiles = []
                    for g in range(MH // 8):
                        ps_g = []
                        for mi in range(8):
                            ps1 = psp.tile([P, TB], f32, tag="ps", bufs=8,
                                           name=f"ps1_{t}_{g}_{mi}")
                            ps_g.append(ps1)
                        for k in range(KD):
                            for mi in range(8):
                                m = g * 8 + mi
                                nc.tensor.matmul(
                                    ps_g[mi][:, :tsz],
                                    w1_sb[k][:, m * P:(m + 1) * P],
                                    xt_sb[k][:, toff:toff + tsz],
                                    start=(k == 0),
                                    stop=(k == KD - 1),
                                )
                        for mi in range(8):
                            m = g * 8 + mi
                            ht = htp.tile([P, TB], mm_dtype, tag=f"ht{m}",
                                          name=f"ht_{t}_{m}")
                            nc.scalar.activation(
                                ht[:, :tsz], ps_g[mi][:, :tsz],
                                mybir.ActivationFunctionType.Relu,
                                bias=b1_sb[:, m:m + 1],
                            )
                            ht_tiles.append(ht)

                    ps2 = psp.tile([C, TB], f32, tag="ps", bufs=8,
                                   name=f"ps2_{t}")
                    for m in range(MH):
                        nc.tensor.matmul(
                            ps2[:, :tsz],
                            w2_sb[:, m * C:(m + 1) * C],
                            ht_tiles[m][:, :tsz],
                            start=(m == 0),
                            stop=(m == MH - 1),
                        )
                    nc.vector.tensor_copy(o2_sb[:, toff:toff + tsz],
                                          ps2[:, :tsz])

                nc.sync.dma_start(ot[:], o2_sb[:])

            if reps == 1:
                body()
            else:
                hints = (mybir.EngineType.PE, mybir.EngineType.SP,
                         mybir.EngineType.Activation, mybir.EngineType.DVE)
                with tc.For_i(0, reps, 1, hint_engines=hints) as iv:
                    body(iv)

    nc.compile()
    return nc


def _get_nc(cap: int):
    key = (cap, MM_DTYPE)
    if key not in _nc_cache:
        _nc_cache[key] = build_nc(cap)
    return _nc_cache[key]


def _expert_mlp_host(xr, W1e, b1e, W2e, b2e):
    h = np.maximum(xr.astype(np.float32) @ W1e + b1e, 0.0)
    return h @ W2e + b2e


def _tf32_round(a: np.ndarray) -> np.ndarray:
    """Round f32 to TF32 (10-bit mantissa), round-to-nearest-even."""
    if MM_DTYPE != mybir.dt.float32r:
        return a
    b = np.ascontiguousarray(a, dtype=np.float32).copy().view(np.uint32)
    b += 0x00000FFF + ((b >> 13) & 1)
    b &= np.uint32(0xFFFFE000)
    return b.view(np.float32)


def make_in_maps(x, W1, b1, W2, idx, cap):
    in_maps = []
    for e in range(E):
        ie = idx[e][:cap]
        xtc = np.zeros((D, cap), dtype=np.float32)
        xtc[:, :len(ie)] = x[ie].T
        in_maps.append({
            "xt": _tf32_round(xtc),
            "w1": _tf32_round(W1[e]),
            "b1t": np.ascontiguousarray(b1[e].reshape(MH, P).T),
            "w2t": _tf32_round(
                W2[e].reshape(MH, P, C).transpose(1, 0, 2).reshape(P, MH * C)),
        })
    return in_maps


def kernel(x, Wr, br, W1, b1, W2, b2):
    x = np.asarray(x, dtype=np.float32)
    Wr = np.asarray(Wr, dtype=np.float32)
    br = np.asarray(br, dtype=np.float32)
    W1 = np.asarray(W1, dtype=np.float32)
    b1 = np.asarray(b1, dtype=np.float32)
    W2 = np.asarray(W2, dtype=np.float32)
    b2 = np.asarray(b2, dtype=np.float32)

    # Router on host: this decides the (expert-parallel) sharding.
    logits = x @ Wr + br
    topics = np.argmax(logits, axis=1)

    idx = [np.flatnonzero(topics == e) for e in range(E)]
    # Keep the NEFF shape fixed; if an expert ever exceeds CAP (astronomically
    # rare for the target distribution) the overflow rows are computed on host.
    cap = CAP
    in_maps = make_in_maps(x, W1, b1, W2, idx, cap)
    nc = _get_nc(cap)
    res = run_bass_kernel_spmd(nc, in_maps, core_ids=list(range(N_CORES)))

    out = np.empty((B, C), dtype=np.float32)
    for e in range(E):
        ie = idx[e][:cap]
        out[ie] = res.results[e]["ot"][:, :len(ie)].T + b2[e]
        if len(idx[e]) > cap:
            ov = idx[e][cap:]
            out[ov] = _expert_mlp_host(x[ov], W1[e], b1[e], W2[e], b2[e])
    return out


# revision 10
# speedup vs baseline: 1.2668x; 1.0183x over previous
"""Expert-mixture (top-1 MoE) Trainium2 kernel, expert-parallel across 8 cores.

Strategy:
  - Host computes the router (x @ Wr + br, argmax) and dispatches tokens:
    all tokens routed to expert e are gathered, transposed, and padded to a
    fixed capacity, forming core e's shard ("all-to-all dispatch by argmax
    topic" done at shard time, since kernel() receives full inputs on host).
  - Core e computes hT = relu(W1[e].T @ xT + b1[e]) followed by
    outT = W2[e].T @ h, entirely on-device (TensorE GEMMs via Tile).
  - Host scatters each expert's rows back into the full [B, C] output and
    adds b2[topic] (bias add commutes with the gather).

Per-core device layout (SPMD, one program):
  xt  [D, CAP]  f32   token block, transposed, zero-padded
  w1  [D, H]    f32   W1[e] (native layout == lhsT chunks)
  b1t [128, 16] f32   b1[e] rearranged so column m = b1[m*128:(m+1)*128]
  w2t [128, 48] f32   W2[e] rearranged so [:, 3m:3m+3] = W2[e][128m:128(m+1)]
  ot  [3, CAP]  f32   output, transposed
"""

import numpy as np

import concourse.bass as bass
import concourse.mybir as mybir
import concourse.tile as tile
from concourse import bacc
from concourse.bass_utils import run_bass_kernel_spmd

B, D, H, E, C = 16384, 1024, 2048, 8, 3
N_CORES = 8
P = 128
KD = D // P    # 8 contraction chunks for GEMM1
MH = H // P    # 16 H chunks
TB = 512       # token block (matmul moving dim)
CAP = 2304     # per-expert token capacity (mean 2048, ~6 sigma headroom)

MM_DTYPE = mybir.dt.float32r  # PE compute dtype (f32 data, full-rate mode)

_nc_cache: dict = {}


def build_nc(cap: int, reps: int = 1, mm_dtype=MM_DTYPE):
    """Build + compile the SPMD program. reps>1 wraps the body in a device
    loop (for steady-state timing); data loads stay inside the loop so each
    iteration models one cold kernel execution."""
    # Token blocks: TB-sized, last may be ragged (min 256 keeps fp32r matmuls
    # at full rate).
    blocks = []
    off = 0
    while off < cap:
        size = min(TB, cap - off)
        blocks.append((off, size))
        off += size
    assert all(s >= 256 for _, s in blocks)

    nc = bacc.Bacc("TRN2", target_bir_lowering=False, debug=False,
                   num_devices=N_CORES)
    f32 = mybir.dt.float32
    # Matmul operands live in mm_dtype end-to-end (DRAM included): for
    # float32r (TF32) the host pre-rounds, so DMA is a pure move and the
    # BIR verifier's "rounded producer" requirement is satisfied at the
    # ExternalInput boundary.
    xt = nc.dram_tensor("xt", [D, cap], mm_dtype, kind="ExternalInput").ap()
    w1 = nc.dram_tensor("w1", [D, H], mm_dtype, kind="ExternalInput").ap()
    b1t = nc.dram_tensor("b1t", [P, MH], f32, kind="ExternalInput").ap()
    w2t = nc.dram_tensor("w2t", [P, MH * C], mm_dtype, kind="ExternalInput").ap()
    ot = nc.dram_tensor("ot", [C, cap], f32, kind="ExternalOutput").ap()

    with tile.TileContext(nc) as tc:
        with (
            tc.tile_pool(name="w1p", bufs=1) as w1p,
            tc.tile_pool(name="xtp", bufs=1) as xtp,
            tc.tile_pool(name="cst", bufs=1) as cst,
            tc.tile_pool(name="htp", bufs=1) as htp,
            tc.tile_pool(name="o2p", bufs=1) as o2p,
            tc.tile_pool(name="ps", bufs=1, space="PSUM") as psp,
        ):
            def body(_iv=None):
                b1_sb = cst.tile([P, MH], f32, tag="b1")
                nc.sync.dma_start(b1_sb[:], b1t[:])
                w2_sb = cst.tile([P, MH * C], mm_dtype, tag="w2")
                nc.sync.dma_start(w2_sb[:], w2t[:])

                # Interleave W1 chunk loads with block-0 xt chunk loads so the
                # k-th GEMM1 step's data arrives in order; later blocks' xt
                # streams during compute (double-buffered per k-chunk).
                w1_sb = []
                xt0_sb = []
                t0sz = blocks[0][1]
                for k in range(KD):
                    wt = w1p.tile([P, H], mm_dtype, tag=f"w1k{k}",
                                  name=f"w1_{k}")
                    nc.sync.dma_start(wt[:], w1[k * P:(k + 1) * P, :])
                    w1_sb.append(wt)
                    xtile = xtp.tile([P, TB], mm_dtype, tag=f"xtk{k}", bufs=2,
                                     name=f"xt_0_{k}")
                    nc.sync.dma_start(xtile[:, :t0sz], xt[k * P:(k + 1) * P,
                                                          0:t0sz])
                    xt0_sb.append(xtile)

                def load_xt_block(t):
                    toff, tsz = blocks[t]
                    if t == 0:
                        return xt0_sb
                    tiles = []
                    for k in range(KD):
                        xtile = xtp.tile([P, TB], mm_dtype, tag=f"xtk{k}",
                                         bufs=2, name=f"xt_{t}_{k}")
                        nc.sync.dma_start(xtile[:, :tsz],
                                          xt[k * P:(k + 1) * P,
                                             toff:toff + tsz])
                        tiles.append(xtile)
                    return tiles

                o2_sb = o2p.tile([C, cap], f32, tag="o2")

                # GEMM1 runs k-outer within groups of 8 H-chunks (8 PSUM
                # banks): the first matmuls only need chunk k=0 of W1/xt, so
                # compute overlaps the remaining weight/activation DMA
                # instead of stalling ~50us on the full 18MB.
                for t, (toff, tsz) in enumerate(blocks):
                    xt_sb = load_xt_block(t)
                    ht_tiles = []
                    for g in range(MH // 8):
                        ps_g = []
                        for mi in range(8):
                            ps1 = psp.tile([P, TB], f32, tag="ps", bufs=8,
                                           name=f"ps1_{t}_{g}_{mi}")
                            ps_g.append(ps1)
                        for k in range(KD):
                            for mi in range(8):
                                m = g * 8 + mi
                                nc.tensor.matmul(
                                    ps_g[mi][:, :tsz],
                                    w1_sb[k][:, m * P:(m + 1) * P],
                                    xt_sb[k][:, :tsz],
                                    start=(k == 0),
                                    stop=(k == KD - 1),
                                )
                        for mi in range(8):
                            m = g * 8 + mi
                            ht = htp.tile([P, TB], mm_dtype, tag=f"ht{m}",
                                          name=f"ht_{t}_{m}")
                            nc.scalar.activation(
                                ht[:, :tsz], ps_g[mi][:, :tsz],
                                mybir.ActivationFunctionType.Relu,
                                bias=b1_sb[:, m:m + 1],
                            )
                            ht_tiles.append(ht)

                    ps2 = psp.tile([C, TB], f32, tag="ps", bufs=8,
                                   name=f"ps2_{t}")
                    for m in range(MH):
                        nc.tensor.matmul(
                            ps2[:, :tsz],
                            w2_sb[:, m * C:(m + 1) * C],
                            ht_tiles[m][:, :tsz],
                            start=(m == 0),
                            stop=(m == MH - 1),
                        )
                    nc.vector.tensor_copy(o2_sb[:, toff:toff + tsz],
                                          ps2[:, :tsz])

                nc.sync.dma_start(ot[:], o2_sb[:])

            if reps == 1:
                body()
            else:
                hints = (mybir.EngineType.PE, mybir.EngineType.SP,
                         mybir.EngineType.Activation, mybir.EngineType.DVE)
                with tc.For_i(0, reps, 1, hint_engines=hints) as iv:
                    body(iv)

    nc.compile()
    return nc


def _get_nc(cap: int):
    key = (cap, MM_DTYPE)
    if key not in _nc_cache:
        _nc_cache[key] = build_nc(cap)
    return _nc_cache[key]


def _expert_mlp_host(xr, W1e, b1e, W2e, b2e):
    h = np.maximum(xr.astype(np.float32) @ W1e + b1e, 0.0)
    return h @ W2e + b2e


def _tf32_round(a: np.ndarray) -> np.ndarray:
    """Round f32 to TF32 (10-bit mantissa), round-to-nearest-even."""
    if MM_DTYPE != mybir.dt.float32r:
        return a
    b = np.ascontiguousarray(a, dtype=np.float32).copy().view(np.uint32)
    b += 0x00000FFF + ((b >> 13) & 1)
    b &= np.uint32(0xFFFFE000)
    return b.view(np.float32)


def make_in_maps(x, W1, b1, W2, idx, cap):
    in_maps = []
    for e in range(E):
        ie = idx[e][:cap]
        xtc = np.zeros((D, cap), dtype=np.float32)
        xtc[:, :len(ie)] = x[ie].T
        in_maps.append({
            "xt": _tf32_round(xtc),
            "w1": _tf32_round(W1[e]),
            "b1t": np.ascontiguousarray(b1[e].reshape(MH, P).T),
            "w2t": _tf32_round(
                W2[e].reshape(MH, P, C).transpose(1, 0, 2).reshape(P, MH * C)),
        })
    return in_maps


def kernel(x, Wr, br, W1, b1, W2, b2):
    x = np.asarray(x, dtype=np.float32)
    Wr = np.asarray(Wr, dtype=np.float32)
    br = np.asarray(br, dtype=np.float32)
    W1 = np.asarray(W1, dtype=np.float32)
    b1 = np.asarray(b1, dtype=np.float32)
    W2 = np.asarray(W2, dtype=np.float32)
    b2 = np.asarray(b2, dtype=np.float32)

    # Router on host: this decides the (expert-parallel) sharding.
    logits = x @ Wr + br
    topics = np.argmax(logits, axis=1)

    idx = [np.flatnonzero(topics == e) for e in range(E)]
    # Keep the NEFF shape fixed; if an expert ever exceeds CAP (astronomically
    # rare for the target distribution) the overflow rows are computed on host.
    cap = CAP
    in_maps = make_in_maps(x, W1, b1, W2, idx, cap)
    nc = _get_nc(cap)
    res = run_bass_kernel_spmd(nc, in_maps, core_ids=list(range(N_CORES)))

    out = np.empty((B, C), dtype=np.float32)
    for e in range(E):
        ie = idx[e][:cap]
        out[ie] = res.results[e]["ot"][:, :len(ie)].T + b2[e]
        if len(idx[e]) > cap:
            ov = idx[e][cap:]
            out[ov] = _expert_mlp_host(x[ov], W1[e], b1[e], W2[e], b2[e])
    return out
